# revision 28
# baseline (speedup 1.0000x reference)
"""AttentionWithMSR Trainium2 kernel — 8-core SPMD, data-parallel over (batch, H-half).

Self-contained: takes FULL inputs, shards internally, returns FULL output.

Math (reference):
    msr  = log1p(x) - (1/3) * sum_s log1p(blur_s(x)),  s in {15, 80, 250}
    a    = BN(conv1x1(g;  Wg)),  b = BN(conv1x1(msr; Wx))
    psi  = sigmoid(BN(conv1x1(relu(a + b); wpsi)))
    out  = x * psi

Kernel mapping:
  * Blurs evaluated on coarse grids only (N=40/12/6 per axis for s=15/80/250):
    Bc = G_s[R,:] @ M @ G_s[:,R] exact at grid points, log1p there, then the
    log field is interpolated back to full res with covariance-optimal
    operators P (rows, own half) and Q (cols). Host-validated error ~4e-6.
  * All three scales' interpolated log fields accumulate into one PSUM tile
    with weight -1/3; msr = log1p(x) + that, computed per channel on-chip.
    One [C, pix] msr field round-trips DRAM for the channel-contracting conv.
  * Conv rhs per pixel-group: rows 0:64 = g, 64:128 = msr; single weight
    matrix w1 = [Wg^T; Wx^T] (BN folded on host). psi conv replicates its
    scalar output over 64 partitions inside the matmul.
"""

import sys

sys.path.insert(0, "/opt/trn_rl_repo")

import numpy as np
import ml_dtypes

SCALES = (15, 80, 250)
EPS = 1e-5
B, C, H, W = 4, 64, 256, 256
HALF = 128
FINT = 32
N_CORES = 8
BF16 = ml_dtypes.bfloat16

# Grid points per scale; blocks padded so every block START is 32-aligned
# (engine AP requirement): s15 rows [0:32], s80 [32:44], s250 [64:70].
NS = [32, 12, 6]
OFFS = [0, 32, 64]
NTOT = 70

_CACHE = {}
_LAST_IN_MAPS = None


def _gauss_mat(scale: int) -> np.ndarray:
    """256x256 matrix of the zero-padded 'same' normalized 1D Gaussian blur."""
    k = int(4 * scale + 1)
    p = k // 2
    coords = np.arange(k, dtype=np.float32) - (k - 1) / 2.0
    g1 = np.exp(-(coords**2) / np.float32(2.0 * scale * scale))
    g1 = g1 / g1.sum()
    i = np.arange(W)
    D = i[None, :] - i[:, None]  # j - i
    M = np.where(np.abs(D) <= p, g1[np.clip(D + p, 0, k - 1)], np.float32(0.0))
    return M.astype(np.float32)


def _build_nc():
    import concourse.mybir as mybir
    import concourse.tile as tile
    from concourse import bacc

    bf = mybir.dt.bfloat16
    f32 = mybir.dt.float32
    AF = mybir.ActivationFunctionType

    nc = bacc.Bacc("TRN2", target_bir_lowering=False)

    # x[b] transposed to (h-chunk, h-in-chunk, channel, w); chunk 0 is this
    # core's OWN half (host reorders), so log1p(x_own) reads chunk 0.
    xbt_e = nc.dram_tensor("xbt", [2, HALF, C, W], bf, kind="ExternalInput")
    # xh2[k*64+c, grp, :] = x[c, grp*2048 + k*1024 + :] (own half)
    xh_e = nc.dram_tensor("xh", [128, 16, 1024], bf, kind="ExternalInput")
    gb_e = nc.dram_tensor("gb", [C, HALF * W], bf, kind="ExternalInput")
    # gsamph[p, ck, OFFS[s]+j] = G_s[rowchunk_ck*128+p, R_s[j]] in the per-core
    # own-first chunk order (pass-A vertical sampling rhs).
    gsamph_e = nc.dram_tensor("gsamph", [128, 2, NTOT], bf, kind="ExternalInput")
    # gsampw: natural w order, j' blocks at JOFFS (zero-padded between).
    gsampw_e = nc.dram_tensor("gsampw", [128, 2, NTOT], bf, kind="ExternalInput")
    # potpad[i, si, p] = P_si[h0+p, i-OFFS_si] when i falls in scale si's row
    # range, else 0 — lets ONE matmul do all 3 scales' row-interp.
    pot_e = nc.dram_tensor("pot", [NTOT, 3, 128], bf, kind="ExternalInput")
    # qtcat[OFFS_s+j, w] = -P_s[w, j]/3 (col-interp, all scales stacked)
    qt_e = nc.dram_tensor("qt", [NTOT, W], bf, kind="ExternalInput")
    w1_e = nc.dram_tensor("w1", [128, FINT], bf, kind="ExternalInput")
    wpsi_e = nc.dram_tensor("wpsi", [128, 64], bf, kind="ExternalInput")
    bias0_e = nc.dram_tensor("bias0", [128, 1], f32, kind="ExternalInput")
    bpsi_e = nc.dram_tensor("bpsi", [128, 1], f32, kind="ExternalInput")
    # out[k*64+c, grp, :] = out_pix[c, grp*2048 + k*1024 + :]
    out_e = nc.dram_tensor("out", [128, 16, 1024], bf, kind="ExternalOutput")

    NPIX = HALF * W  # 32768 pixels per core

    with tile.TileContext(nc) as tc:
        with (
            tc.tile_pool(name="consts", bufs=1) as consts,
            tc.tile_pool(name="dram", bufs=1, space="DRAM") as dpool,
        ):
            msrd = dpool.tile([C, NPIX], bf)
            msrd_v = msrd[:].rearrange("c (h w) -> c h w", h=HALF)

            # ---- phase 1: coarse blur sampling + log1p + interp expansion,
            # software-pipelined over channels. xbt loads go FIRST on the
            # sync queue so channel 0 is ready asap; consts ride scalar.
            with (
                tc.tile_pool(name="p1", bufs=4) as p1,
                tc.tile_pool(name="p1x", bufs=8) as p1x,
                tc.tile_pool(name="p1va", bufs=2, space="PSUM") as p1va,
                tc.tile_pool(name="p1lp", bufs=2, space="PSUM") as p1lp,
                tc.tile_pool(name="p1rt", bufs=2, space="PSUM") as p1rt,
                tc.tile_pool(name="p1e", bufs=2, space="PSUM") as p1e,
            ):
                GRP = 8  # channels per staged load
                xs_tiles = {}

                def load_group(g):
                    xs_g = p1x.tile([128, 2, GRP, W], bf, tag="xs")
                    for hc in range(2):
                        nc.sync.dma_start(
                            xs_g[:, hc], xbt_e[hc, :, g * GRP : (g + 1) * GRP, :]
                        )
                    xs_tiles[g] = xs_g

                for g in range(C // GRP):
                    load_group(g)

                gsamph_sb = consts.tile([128, 2, NTOT], bf)
                nc.scalar.dma_start(gsamph_sb[:], gsamph_e[:])
                gsampw_sb = consts.tile([128, 2, NTOT], bf)
                nc.scalar.dma_start(gsampw_sb[:], gsampw_e[:])
                pot_sb = consts.tile([NTOT, 3, 128], bf)
                nc.scalar.dma_start(pot_sb[:], pot_e[:])
                qt_sb = consts.tile([NTOT, W], bf)
                nc.scalar.dma_start(qt_sb[:], qt_e[:])
                w1_sb = consts.tile([128, FINT], bf)
                nc.scalar.dma_start(w1_sb[:], w1_e[:])
                wpsi_sb = consts.tile([128, 64], bf)
                nc.scalar.dma_start(wpsi_sb[:], wpsi_e[:])
                bias0_sb = consts.tile([128, 1], f32)
                nc.scalar.dma_start(bias0_sb[:], bias0_e[:])
                bpsi_sb = consts.tile([128, 1], f32)
                nc.scalar.dma_start(bpsi_sb[:], bpsi_e[:])

                vt_tiles = {}
                lc_tiles = {}
                rt_tiles = {}
                msr_pair = [None]

                def pass_a(c):
                    # vertical coarse sampling: vt[w, j] = sum_h x[h, w] G_s[h, R_s[j]]
                    xs_g = xs_tiles[c // GRP]
                    ci = c % GRP
                    va_ps = p1va.tile([128, 2, NTOT], f32, tag="vaps")
                    for wc in range(2):
                        for hc in range(2):
                            nc.tensor.matmul(
                                va_ps[:, wc, :],
                                lhsT=xs_g[:, hc, ci, wc * 128 : (wc + 1) * 128],
                                rhs=gsamph_sb[:, hc, :],
                                start=(hc == 0),
                                stop=(hc == 1),
                            )
                    vt_sb = p1.tile([128, 2, NTOT], bf, tag="vt")
                    nc.vector.tensor_copy(vt_sb[:], va_ps[:])
                    vt_tiles[c] = vt_sb

                def pass_b(c):
                    # horizontal coarse sampling + log1p over ALL scales in 2
                    # matmuls (cross-scale cells are junk, never read).
                    vt_sb = vt_tiles.pop(c)
                    lp_ps = p1lp.tile([NTOT, NTOT], f32, tag="lpps")
                    for wc in range(2):
                        nc.tensor.matmul(
                            lp_ps[:],
                            lhsT=vt_sb[:, wc, :],
                            rhs=gsampw_sb[:, wc, :],
                            start=(wc == 0),
                            stop=(wc == 1),
                        )
                    lc_sb = p1.tile([NTOT, NTOT], bf, tag="lc")
                    nc.scalar.activation(lc_sb[:], lp_ps[:], AF.Ln, bias=1.0)
                    lc_tiles[c] = lc_sb

                def expand_v(c):
                    # row-interp: one matmul per scale, col-packed so the
                    # scale blocks land at aligned partition offsets of ONE
                    # [NTOT, 128] tile (single PSUM->SBUF copy).
                    lc_sb = lc_tiles.pop(c)
                    rt_ps = p1rt.tile([NTOT, 128], f32, tag="rtps")
                    for si in range(3):
                        off, n = OFFS[si], NS[si]
                        nc.tensor.matmul(
                            rt_ps[off : off + n, :],
                            lhsT=lc_sb[:, off : off + n],
                            rhs=pot_sb[:, si, :],
                            start=True,
                            stop=True,
                            tile_position=(0, off),
                        )
                    rt_sb = p1.tile([NTOT, 128], bf, tag="rt")
                    nc.vector.tensor_copy(rt_sb[:], rt_ps[:])
                    rt_tiles[c] = rt_sb

                def expand_h(c):
                    # col-interp + scale-sum in ONE matmul (qtcat carries the
                    # -1/3); then msr = log1p(x_own) + m3.
                    rt_sb = rt_tiles.pop(c)
                    m3_ps = p1e.tile([128, W], f32, tag="m3ps")
                    nc.tensor.matmul(
                        m3_ps[:],
                        lhsT=rt_sb[:],
                        rhs=qt_sb[:],
                        start=True,
                        stop=True,
                    )
                    xs_g = xs_tiles[c // GRP]
                    ci = c % GRP
                    lx_sb = p1.tile([128, W], bf, tag="lx")
                    nc.scalar.activation(lx_sb[:], xs_g[:, 0, ci, :], AF.Ln, bias=1.0)
                    if c % 2 == 0:
                        msr_new = p1.tile([128, 2, W], bf, tag="msr")
                        msr_pair[0] = msr_new
                    msr_sb = msr_pair[0]
                    nc.vector.tensor_add(msr_sb[:, c % 2, :], lx_sb[:], m3_ps[:])
                    if c % 2 == 1:
                        nc.gpsimd.dma_start(
                            msrd_v[c - 1 : c + 1].rearrange("c h w -> h c w"),
                            msr_sb[:],
                        )

                # 4-deep stage-offset pipeline over channels.
                for c in range(4):
                    pass_a(c)
                pass_b(0)
                pass_b(1)
                expand_v(0)
                for c in range(C):
                    if c + 4 < C:
                        pass_a(c + 4)
                    if c + 2 < C:
                        pass_b(c + 2)
                    if c + 1 < C:
                        expand_v(c + 1)
                    expand_h(c)

            # ---- phase 2: conv1x1s + relu + psi + sigmoid + multiply
            with (
                tc.tile_pool(name="p2", bufs=2) as p2,
                tc.tile_pool(name="p2r", bufs=3) as p2r,
                tc.tile_pool(name="p2x", bufs=3) as p2x,
                tc.tile_pool(name="p2ab", bufs=2, space="PSUM") as p2ab,
                tc.tile_pool(name="p2s", bufs=2, space="PSUM") as p2s,
            ):
                rhs_tiles = {}
                xb2_tiles = {}

                def gather_rhs(grp):
                    r = p2r.tile([128, GRP, W], bf, tag="rhs")
                    px = grp * 2048
                    nc.sync.dma_start(r[0:64], gb_e[:, px : px + 2048])
                    nc.sync.dma_start(r[64:128], msrd[:, px : px + 2048])
                    rhs_tiles[grp] = r

                def load_xb2(grp):
                    xb2 = p2x.tile([128, 1024], bf, tag="xb2")
                    nc.sync.dma_start(xb2[:], xh_e[:, grp, :])
                    xb2_tiles[grp] = xb2

                ab_tiles = {}

                def do_ab(grp):
                    rhs = rhs_tiles.pop(grp)
                    rhsf = rhs[:].rearrange("p h w -> p (h w)")
                    ab_ps = p2ab.tile([128, 512], f32, tag="abps")
                    for t in range(4):
                        nc.tensor.matmul(
                            ab_ps[32 * t : 32 * t + 32, :],
                            lhsT=w1_sb[:],
                            rhs=rhsf[:, 512 * t : 512 * (t + 1)],
                            start=True,
                            stop=True,
                            tile_position=(0, 32 * t),
                        )
                    ab_tiles[grp] = ab_ps

                gather_rhs(0)
                gather_rhs(1)
                load_xb2(0)
                load_xb2(1)
                do_ab(0)
                for grp in range(16):
                    if grp + 2 < 16:
                        gather_rhs(grp + 2)
                        load_xb2(grp + 2)
                    if grp + 1 < 16:
                        do_ab(grp + 1)
                    px = grp * 2048
                    ab_ps = ab_tiles.pop(grp)
                    relu_sb = p2.tile([128, 512], bf, tag="relu")
                    nc.vector.tensor_scalar(
                        relu_sb[:],
                        ab_ps[:],
                        bias0_sb[:],
                        0.0,
                        mybir.AluOpType.add,
                        mybir.AluOpType.max,
                    )
                    s_ps = p2s.tile([128, 1024], f32, tag="sps")
                    for t in range(4):
                        a, bb = t // 2, t % 2
                        nc.tensor.matmul(
                            s_ps[64 * a : 64 * a + 64, 512 * bb : 512 * bb + 512],
                            lhsT=wpsi_sb[32 * t : 32 * t + 32, :],
                            rhs=relu_sb[32 * t : 32 * t + 32, :],
                            start=True,
                            stop=True,
                            tile_position=(32 * t, 64 * a),
                        )
                    psi_sb = p2.tile([128, 1024], bf, tag="psi")
                    nc.scalar.activation(
                        psi_sb[:], s_ps[:], AF.Sigmoid, bias=bpsi_sb[:]
                    )
                    xb2 = xb2_tiles.pop(grp)
                    out2 = p2.tile([128, 1024], bf, tag="out2")
                    nc.gpsimd.tensor_mul(out2[:], xb2[:], psi_sb[:])
                    nc.gpsimd.dma_start(out_e[:, grp, :], out2[:])

    nc.finalize()
    return nc


def kernel(**inputs):
    from concourse.bass_utils import run_bass_kernel_spmd

    g = np.asarray(inputs["g"], dtype=np.float32)
    x = np.asarray(inputs["x"], dtype=np.float32)

    def f(name):
        return np.asarray(inputs[name], dtype=np.float32)

    # Fold eval-mode BN into the 1x1 convs.
    ag = f("wg_gamma") / np.sqrt(f("wg_var") + EPS)
    wg_eff = ag[:, None] * f("wg_w")[:, :, 0, 0]  # [32, 64]
    bg_eff = ag * (f("wg_b") - f("wg_mean")) + f("wg_beta")
    ax = f("wx_gamma") / np.sqrt(f("wx_var") + EPS)
    wx_eff = ax[:, None] * f("wx_w")[:, :, 0, 0]  # [32, 64]
    bx_eff = ax * (f("wx_b") - f("wx_mean")) + f("wx_beta")
    ap_ = f("psi_gamma") / np.sqrt(f("psi_var") + EPS)
    wpsi_eff = ap_[0] * f("psi_w")[0, :, 0, 0]  # [32]
    bpsi = float(ap_[0] * (f("psi_b")[0] - f("psi_mean")[0]) + f("psi_beta")[0])
    bias0 = bg_eff + bx_eff  # [32]

    Gs = [_gauss_mat(s).astype(np.float64) for s in SCALES]

    # Per-scale grids + covariance-optimal log-field interpolators.
    grids, Ps = [], []
    for G, n in zip(Gs, NS):
        grid = np.unique(np.round(np.linspace(0, W - 1, n)).astype(int))
        assert len(grid) == n
        C2 = G @ G.T
        Ps.append(
            C2[:, grid]
            @ np.linalg.pinv(C2[np.ix_(grid, grid)], rcond=1e-6, hermitian=True)
        )  # [256, n]
        grids.append(grid)

    # gsamp[p, ck, OFFS[si]+j] = G_si[ck*128+p, grids[si][j]] (pads stay 0)
    gsamp = np.zeros((128, 2, NTOT), dtype=np.float64)
    for si in range(3):
        for ck in range(2):
            gsamp[:, ck, OFFS[si] : OFFS[si] + NS[si]] = Gs[si][
                ck * 128 : (ck + 1) * 128
            ][:, grids[si]]
    gsampw_bf = gsamp.astype(BF16)

    # qtcat[OFFS[si]+j, w] = -P_si[w, j]/3 (pad rows stay 0)
    qt = np.zeros((NTOT, W), dtype=np.float64)
    for si in range(3):
        qt[OFFS[si] : OFFS[si] + NS[si], :] = -Ps[si].T / 3.0
    qt = qt.astype(BF16)

    # w1: rows 0-63 -> g channels (Wg), 64-127 -> msr channels (Wx)
    w1 = np.concatenate([wg_eff.T, wx_eff.T], axis=0).astype(BF16)  # [128, 32]
    wpsi_t = np.broadcast_to(
        np.tile(wpsi_eff, 4)[:, None], (128, 64)
    ).astype(BF16)  # [128, 64]: row 32t+o = wpsi[o], replicated over 64 cols
    bias0_t = np.tile(bias0, 4)[:, None].astype(np.float32)  # [128, 1]
    bpsi_t = np.full((128, 1), bpsi, dtype=np.float32)

    key = "nc"
    if key not in _CACHE:
        _CACHE[key] = _build_nc()
    nc = _CACHE[key]

    in_maps = []
    for core in range(N_CORES):
        b, half = core // 2, core % 2
        h0 = half * HALF
        # potpad[i, si, p] = P_si[h0+p, i-OFFS_si] inside scale si's rows, else 0
        pot = np.zeros((NTOT, 3, 128), dtype=np.float64)
        for si in range(3):
            pot[OFFS[si] : OFFS[si] + NS[si], si, :] = Ps[si][h0 : h0 + HALF, :].T
        # gsamph: own-half chunk first (matches xbt chunk reorder)
        gsamph = gsamp[:, [half, 1 - half], :].astype(BF16)
        in_maps.append(
            {
                "xbt": np.ascontiguousarray(
                    x[b].reshape(C, 2, HALF, W)[:, [half, 1 - half]]
                    .transpose(1, 2, 0, 3)
                ).astype(BF16),
                "xh": np.ascontiguousarray(
                    x[b, :, h0 : h0 + HALF, :]
                    .reshape(C, 16, 2, 1024)
                    .transpose(2, 0, 1, 3)
                    .reshape(128, 16, 1024)
                ).astype(BF16),
                "gb": g[b, :, h0 : h0 + HALF, :].reshape(C, HALF * W).astype(BF16),
                "gsamph": gsamph,
                "gsampw": gsampw_bf,
                "pot": pot.astype(BF16),
                "qt": qt,
                "w1": w1,
                "wpsi": wpsi_t,
                "bias0": bias0_t,
                "bpsi": bpsi_t,
            }
        )

    global _LAST_IN_MAPS
    _LAST_IN_MAPS = in_maps
    res = run_bass_kernel_spmd(nc, in_maps, core_ids=list(range(N_CORES)))

    out = np.empty((B, C, H, W), dtype=np.float32)
    for core in range(N_CORES):
        b, half = core // 2, core % 2
        h0 = half * HALF
        o = res.results[core]["out"].astype(np.float32)  # [128, 16, 1024]
        o = o.reshape(2, 64, 16, 1024).transpose(1, 2, 0, 3).reshape(C, HALF, W)
        out[b, :, h0 : h0 + HALF, :] = o
    return out


# revision 30
# speedup vs baseline: 1.1812x; 1.1812x over previous
"""AttentionWithMSR Trainium2 kernel — 8-core SPMD, data-parallel over (batch, H-half).

Self-contained: takes FULL inputs, shards internally, returns FULL output.

Math (reference):
    msr  = log1p(x) - (1/3) * sum_s log1p(blur_s(x)),  s in {15, 80, 250}
    a    = BN(conv1x1(g;  Wg)),  b = BN(conv1x1(msr; Wx))
    psi  = sigmoid(BN(conv1x1(relu(a + b); wpsi)))
    out  = x * psi

Kernel mapping:
  * Blurs evaluated on coarse grids only (N=40/12/6 per axis for s=15/80/250):
    Bc = G_s[R,:] @ M @ G_s[:,R] exact at grid points, log1p there, then the
    log field is interpolated back to full res with covariance-optimal
    operators P (rows, own half) and Q (cols). Host-validated error ~4e-6.
  * All three scales' interpolated log fields accumulate into one PSUM tile
    with weight -1/3; msr = log1p(x) + that, computed per channel on-chip.
    One [C, pix] msr field round-trips DRAM for the channel-contracting conv.
  * Conv rhs per pixel-group: rows 0:64 = g, 64:128 = msr; single weight
    matrix w1 = [Wg^T; Wx^T] (BN folded on host). psi conv replicates its
    scalar output over 64 partitions inside the matmul.
"""

import sys

sys.path.insert(0, "/opt/trn_rl_repo")

import numpy as np
import ml_dtypes

SCALES = (15, 80, 250)
EPS = 1e-5
B, C, H, W = 4, 64, 256, 256
HALF = 128
FINT = 32
N_CORES = 8
BF16 = ml_dtypes.bfloat16

# Grid points per scale; blocks padded so every block START is 32-aligned
# (engine AP requirement): s15 rows [0:32], s80 [32:44], s250 [64:70].
NS = [32, 12, 6]
OFFS = [0, 32, 64]
NTOT = 70

_CACHE = {}
_LAST_IN_MAPS = None


def _gauss_mat(scale: int) -> np.ndarray:
    """256x256 matrix of the zero-padded 'same' normalized 1D Gaussian blur."""
    k = int(4 * scale + 1)
    p = k // 2
    coords = np.arange(k, dtype=np.float32) - (k - 1) / 2.0
    g1 = np.exp(-(coords**2) / np.float32(2.0 * scale * scale))
    g1 = g1 / g1.sum()
    i = np.arange(W)
    D = i[None, :] - i[:, None]  # j - i
    M = np.where(np.abs(D) <= p, g1[np.clip(D + p, 0, k - 1)], np.float32(0.0))
    return M.astype(np.float32)


def _build_nc():
    import concourse.mybir as mybir
    import concourse.tile as tile
    from concourse import bacc

    bf = mybir.dt.bfloat16
    f32 = mybir.dt.float32
    AF = mybir.ActivationFunctionType

    nc = bacc.Bacc("TRN2", target_bir_lowering=False)

    # x[b] transposed to (h-chunk, h-in-chunk, channel, w); chunk 0 is this
    # core's OWN half (host reorders), so log1p(x_own) reads chunk 0.
    xbt_e = nc.dram_tensor("xbt", [2, HALF, C, W], bf, kind="ExternalInput")
    # xh2[k*64+c, grp, :] = x[c, grp*2048 + k*1024 + :] (own half)
    xh_e = nc.dram_tensor("xh", [128, 16, 1024], bf, kind="ExternalInput")
    gb_e = nc.dram_tensor("gb", [C, HALF * W], bf, kind="ExternalInput")
    # gsamph[p, ck, OFFS[s]+j] = G_s[rowchunk_ck*128+p, R_s[j]] in the per-core
    # own-first chunk order (pass-A vertical sampling rhs).
    gsamph_e = nc.dram_tensor("gsamph", [128, 2, NTOT], bf, kind="ExternalInput")
    # gsampw: natural w order, j' blocks at JOFFS (zero-padded between).
    gsampw_e = nc.dram_tensor("gsampw", [128, 2, NTOT], bf, kind="ExternalInput")
    # potpad[i, si, p] = P_si[h0+p, i-OFFS_si] when i falls in scale si's row
    # range, else 0 — lets ONE matmul do all 3 scales' row-interp.
    pot_e = nc.dram_tensor("pot", [NTOT, 3, 128], bf, kind="ExternalInput")
    # qtcat[OFFS_s+j, w] = -P_s[w, j]/3 (col-interp, all scales stacked)
    qt_e = nc.dram_tensor("qt", [NTOT, W], bf, kind="ExternalInput")
    w1_e = nc.dram_tensor("w1", [128, FINT], bf, kind="ExternalInput")
    wpsi_e = nc.dram_tensor("wpsi", [128, 64], bf, kind="ExternalInput")
    bias0_e = nc.dram_tensor("bias0", [128, 1], f32, kind="ExternalInput")
    bpsi_e = nc.dram_tensor("bpsi", [128, 1], f32, kind="ExternalInput")
    # out[k*64+c, grp, :] = out_pix[c, grp*2048 + k*1024 + :]
    out_e = nc.dram_tensor("out", [128, 16, 1024], bf, kind="ExternalOutput")

    NPIX = HALF * W  # 32768 pixels per core

    with tile.TileContext(nc) as tc:
        with (
            tc.tile_pool(name="consts", bufs=1) as consts,
            tc.tile_pool(name="dram", bufs=1, space="DRAM") as dpool,
        ):
            msrd = dpool.tile([C, NPIX], bf)
            msrd_v = msrd[:].rearrange("c (h w) -> c h w", h=HALF)

            # ---- phase 1: coarse blur sampling + log1p + interp expansion,
            # software-pipelined over channels. xbt loads go FIRST on the
            # sync queue so channel 0 is ready asap; consts ride scalar.
            with (
                tc.tile_pool(name="p1", bufs=4) as p1,
                tc.tile_pool(name="p1x", bufs=8) as p1x,
                tc.tile_pool(name="p1va", bufs=2, space="PSUM") as p1va,
                tc.tile_pool(name="p1lp", bufs=2, space="PSUM") as p1lp,
                tc.tile_pool(name="p1rt", bufs=2, space="PSUM") as p1rt,
                tc.tile_pool(name="p1e", bufs=2, space="PSUM") as p1e,
            ):
                GRP = 8  # channels per staged load
                xs_tiles = {}

                def load_group(g):
                    xs_g = p1x.tile([128, 2, GRP, W], bf, tag="xs")
                    for hc in range(2):
                        nc.sync.dma_start(
                            xs_g[:, hc], xbt_e[hc, :, g * GRP : (g + 1) * GRP, :]
                        )
                    xs_tiles[g] = xs_g

                for g in range(C // GRP):
                    load_group(g)

                gsamph_sb = consts.tile([128, 2, NTOT], bf)
                nc.scalar.dma_start(gsamph_sb[:], gsamph_e[:])
                gsampw_sb = consts.tile([128, 2, NTOT], bf)
                nc.scalar.dma_start(gsampw_sb[:], gsampw_e[:])
                pot_sb = consts.tile([NTOT, 3, 128], bf)
                nc.scalar.dma_start(pot_sb[:], pot_e[:])
                qt_sb = consts.tile([NTOT, W], bf)
                nc.scalar.dma_start(qt_sb[:], qt_e[:])
                w1_sb = consts.tile([128, FINT], bf)
                nc.scalar.dma_start(w1_sb[:], w1_e[:])
                wpsi_sb = consts.tile([128, 64], bf)
                nc.scalar.dma_start(wpsi_sb[:], wpsi_e[:])
                bias0_sb = consts.tile([128, 1], f32)
                nc.scalar.dma_start(bias0_sb[:], bias0_e[:])
                bpsi_sb = consts.tile([128, 1], f32)
                nc.scalar.dma_start(bpsi_sb[:], bpsi_e[:])

                vt_tiles = {}
                lc_tiles = {}
                rt_tiles = {}
                msr_pair = [None]

                def pass_a(c):
                    # vertical coarse sampling: vt[w, j] = sum_h x[h, w] G_s[h, R_s[j]]
                    xs_g = xs_tiles[c // GRP]
                    ci = c % GRP
                    va_ps = p1va.tile([128, 2, NTOT], f32, tag="vaps")
                    for wc in range(2):
                        for hc in range(2):
                            nc.tensor.matmul(
                                va_ps[:, wc, :],
                                lhsT=xs_g[:, hc, ci, wc * 128 : (wc + 1) * 128],
                                rhs=gsamph_sb[:, hc, :],
                                start=(hc == 0),
                                stop=(hc == 1),
                            )
                    vt_sb = p1.tile([128, 2, NTOT], bf, tag="vt")
                    nc.vector.tensor_copy(vt_sb[:], va_ps[:])
                    vt_tiles[c] = vt_sb

                def pass_b(c):
                    # horizontal coarse sampling + log1p over ALL scales in 2
                    # matmuls (cross-scale cells are junk, never read).
                    vt_sb = vt_tiles.pop(c)
                    lp_ps = p1lp.tile([NTOT, NTOT], f32, tag="lpps")
                    for wc in range(2):
                        nc.tensor.matmul(
                            lp_ps[:],
                            lhsT=vt_sb[:, wc, :],
                            rhs=gsampw_sb[:, wc, :],
                            start=(wc == 0),
                            stop=(wc == 1),
                        )
                    lc_sb = p1.tile([NTOT, NTOT], bf, tag="lc")
                    nc.scalar.activation(lc_sb[:], lp_ps[:], AF.Ln, bias=1.0)
                    lc_tiles[c] = lc_sb

                def expand_v(c):
                    # row-interp: one matmul per scale, col-packed so the
                    # scale blocks land at aligned partition offsets of ONE
                    # [NTOT, 128] tile (single PSUM->SBUF copy).
                    lc_sb = lc_tiles.pop(c)
                    rt_ps = p1rt.tile([NTOT, 128], f32, tag="rtps")
                    for si in range(3):
                        off, n = OFFS[si], NS[si]
                        nc.tensor.matmul(
                            rt_ps[off : off + n, :],
                            lhsT=lc_sb[:, off : off + n],
                            rhs=pot_sb[:, si, :],
                            start=True,
                            stop=True,
                            tile_position=(0, off),
                        )
                    rt_sb = p1.tile([NTOT, 128], bf, tag="rt")
                    nc.vector.tensor_copy(rt_sb[:], rt_ps[:])
                    rt_tiles[c] = rt_sb

                def expand_h(c):
                    # col-interp + scale-sum in ONE matmul (qtcat carries the
                    # -1/3); then msr = log1p(x_own) + m3.
                    rt_sb = rt_tiles.pop(c)
                    m3_ps = p1e.tile([128, W], f32, tag="m3ps")
                    nc.tensor.matmul(
                        m3_ps[:],
                        lhsT=rt_sb[:],
                        rhs=qt_sb[:],
                        start=True,
                        stop=True,
                    )
                    xs_g = xs_tiles[c // GRP]
                    ci = c % GRP
                    lx_sb = p1.tile([128, W], bf, tag="lx")
                    nc.scalar.activation(lx_sb[:], xs_g[:, 0, ci, :], AF.Ln, bias=1.0)
                    if c % 4 == 0:
                        msr_new = p1.tile([128, 4, W], bf, tag="msr")
                        msr_pair[0] = msr_new
                    msr_sb = msr_pair[0]
                    nc.vector.tensor_add(msr_sb[:, c % 4, :], lx_sb[:], m3_ps[:])
                    if c % 4 == 3:
                        nc.gpsimd.dma_start(
                            msrd_v[c - 3 : c + 1].rearrange("c h w -> h c w"),
                            msr_sb[:],
                        )

                # 6-deep stage-offset pipeline over channels.
                for c in range(6):
                    pass_a(c)
                pass_b(0)
                pass_b(1)
                pass_b(2)
                pass_b(3)
                expand_v(0)
                expand_v(1)
                for c in range(C):
                    if c + 6 < C:
                        pass_a(c + 6)
                    if c + 4 < C:
                        pass_b(c + 4)
                    if c + 2 < C:
                        expand_v(c + 2)
                    expand_h(c)

            # ---- phase 2: conv1x1s + relu + psi + sigmoid + multiply
            with (
                tc.tile_pool(name="p2", bufs=2) as p2,
                tc.tile_pool(name="p2r", bufs=3) as p2r,
                tc.tile_pool(name="p2x", bufs=3) as p2x,
                tc.tile_pool(name="p2ab", bufs=2, space="PSUM") as p2ab,
                tc.tile_pool(name="p2s", bufs=2, space="PSUM") as p2s,
            ):
                rhs_tiles = {}
                xb2_tiles = {}

                def gather_rhs(grp):
                    r = p2r.tile([128, GRP, W], bf, tag="rhs")
                    px = grp * 2048
                    nc.sync.dma_start(r[0:64], gb_e[:, px : px + 2048])
                    nc.sync.dma_start(r[64:128], msrd[:, px : px + 2048])
                    rhs_tiles[grp] = r

                def load_xb2(grp):
                    xb2 = p2x.tile([128, 1024], bf, tag="xb2")
                    nc.sync.dma_start(xb2[:], xh_e[:, grp, :])
                    xb2_tiles[grp] = xb2

                ab_tiles = {}

                def do_ab(grp):
                    rhs = rhs_tiles.pop(grp)
                    rhsf = rhs[:].rearrange("p h w -> p (h w)")
                    ab_ps = p2ab.tile([128, 512], f32, tag="abps")
                    for t in range(4):
                        nc.tensor.matmul(
                            ab_ps[32 * t : 32 * t + 32, :],
                            lhsT=w1_sb[:],
                            rhs=rhsf[:, 512 * t : 512 * (t + 1)],
                            start=True,
                            stop=True,
                            tile_position=(0, 32 * t),
                        )
                    ab_tiles[grp] = ab_ps

                gather_rhs(0)
                gather_rhs(1)
                load_xb2(0)
                load_xb2(1)
                do_ab(0)
                for grp in range(16):
                    if grp + 2 < 16:
                        gather_rhs(grp + 2)
                        load_xb2(grp + 2)
                    if grp + 1 < 16:
                        do_ab(grp + 1)
                    px = grp * 2048
                    ab_ps = ab_tiles.pop(grp)
                    relu_sb = p2.tile([128, 512], bf, tag="relu")
                    nc.vector.tensor_scalar(
                        relu_sb[:],
                        ab_ps[:],
                        bias0_sb[:],
                        0.0,
                        mybir.AluOpType.add,
                        mybir.AluOpType.max,
                    )
                    s_ps = p2s.tile([128, 1024], f32, tag="sps")
                    for t in range(4):
                        a, bb = t // 2, t % 2
                        nc.tensor.matmul(
                            s_ps[64 * a : 64 * a + 64, 512 * bb : 512 * bb + 512],
                            lhsT=wpsi_sb[32 * t : 32 * t + 32, :],
                            rhs=relu_sb[32 * t : 32 * t + 32, :],
                            start=True,
                            stop=True,
                            tile_position=(32 * t, 64 * a),
                        )
                    psi_sb = p2.tile([128, 1024], bf, tag="psi")
                    nc.scalar.activation(
                        psi_sb[:], s_ps[:], AF.Sigmoid, bias=bpsi_sb[:]
                    )
                    xb2 = xb2_tiles.pop(grp)
                    out2 = p2.tile([128, 1024], bf, tag="out2")
                    nc.gpsimd.tensor_mul(out2[:], xb2[:], psi_sb[:])
                    nc.gpsimd.dma_start(out_e[:, grp, :], out2[:])

    nc.finalize()
    return nc


def kernel(**inputs):
    from concourse.bass_utils import run_bass_kernel_spmd

    g = np.asarray(inputs["g"], dtype=np.float32)
    x = np.asarray(inputs["x"], dtype=np.float32)

    def f(name):
        return np.asarray(inputs[name], dtype=np.float32)

    # Fold eval-mode BN into the 1x1 convs.
    ag = f("wg_gamma") / np.sqrt(f("wg_var") + EPS)
    wg_eff = ag[:, None] * f("wg_w")[:, :, 0, 0]  # [32, 64]
    bg_eff = ag * (f("wg_b") - f("wg_mean")) + f("wg_beta")
    ax = f("wx_gamma") / np.sqrt(f("wx_var") + EPS)
    wx_eff = ax[:, None] * f("wx_w")[:, :, 0, 0]  # [32, 64]
    bx_eff = ax * (f("wx_b") - f("wx_mean")) + f("wx_beta")
    ap_ = f("psi_gamma") / np.sqrt(f("psi_var") + EPS)
    wpsi_eff = ap_[0] * f("psi_w")[0, :, 0, 0]  # [32]
    bpsi = float(ap_[0] * (f("psi_b")[0] - f("psi_mean")[0]) + f("psi_beta")[0])
    bias0 = bg_eff + bx_eff  # [32]

    Gs = [_gauss_mat(s).astype(np.float64) for s in SCALES]

    # Per-scale grids + covariance-optimal log-field interpolators.
    grids, Ps = [], []
    for G, n in zip(Gs, NS):
        grid = np.unique(np.round(np.linspace(0, W - 1, n)).astype(int))
        assert len(grid) == n
        C2 = G @ G.T
        Ps.append(
            C2[:, grid]
            @ np.linalg.pinv(C2[np.ix_(grid, grid)], rcond=1e-6, hermitian=True)
        )  # [256, n]
        grids.append(grid)

    # gsamp[p, ck, OFFS[si]+j] = G_si[ck*128+p, grids[si][j]] (pads stay 0)
    gsamp = np.zeros((128, 2, NTOT), dtype=np.float64)
    for si in range(3):
        for ck in range(2):
            gsamp[:, ck, OFFS[si] : OFFS[si] + NS[si]] = Gs[si][
                ck * 128 : (ck + 1) * 128
            ][:, grids[si]]
    gsampw_bf = gsamp.astype(BF16)

    # qtcat[OFFS[si]+j, w] = -P_si[w, j]/3 (pad rows stay 0)
    qt = np.zeros((NTOT, W), dtype=np.float64)
    for si in range(3):
        qt[OFFS[si] : OFFS[si] + NS[si], :] = -Ps[si].T / 3.0
    qt = qt.astype(BF16)

    # w1: rows 0-63 -> g channels (Wg), 64-127 -> msr channels (Wx)
    w1 = np.concatenate([wg_eff.T, wx_eff.T], axis=0).astype(BF16)  # [128, 32]
    wpsi_t = np.broadcast_to(
        np.tile(wpsi_eff, 4)[:, None], (128, 64)
    ).astype(BF16)  # [128, 64]: row 32t+o = wpsi[o], replicated over 64 cols
    bias0_t = np.tile(bias0, 4)[:, None].astype(np.float32)  # [128, 1]
    bpsi_t = np.full((128, 1), bpsi, dtype=np.float32)

    key = "nc"
    if key not in _CACHE:
        _CACHE[key] = _build_nc()
    nc = _CACHE[key]

    in_maps = []
    for core in range(N_CORES):
        b, half = core // 2, core % 2
        h0 = half * HALF
        # potpad[i, si, p] = P_si[h0+p, i-OFFS_si] inside scale si's rows, else 0
        pot = np.zeros((NTOT, 3, 128), dtype=np.float64)
        for si in range(3):
            pot[OFFS[si] : OFFS[si] + NS[si], si, :] = Ps[si][h0 : h0 + HALF, :].T
        # gsamph: own-half chunk first (matches xbt chunk reorder)
        gsamph = gsamp[:, [half, 1 - half], :].astype(BF16)
        in_maps.append(
            {
                "xbt": np.ascontiguousarray(
                    x[b].reshape(C, 2, HALF, W)[:, [half, 1 - half]]
                    .transpose(1, 2, 0, 3)
                ).astype(BF16),
                "xh": np.ascontiguousarray(
                    x[b, :, h0 : h0 + HALF, :]
                    .reshape(C, 16, 2, 1024)
                    .transpose(2, 0, 1, 3)
                    .reshape(128, 16, 1024)
                ).astype(BF16),
                "gb": g[b, :, h0 : h0 + HALF, :].reshape(C, HALF * W).astype(BF16),
                "gsamph": gsamph,
                "gsampw": gsampw_bf,
                "pot": pot.astype(BF16),
                "qt": qt,
                "w1": w1,
                "wpsi": wpsi_t,
                "bias0": bias0_t,
                "bpsi": bpsi_t,
            }
        )

    global _LAST_IN_MAPS
    _LAST_IN_MAPS = in_maps
    res = run_bass_kernel_spmd(nc, in_maps, core_ids=list(range(N_CORES)))

    out = np.empty((B, C, H, W), dtype=np.float32)
    for core in range(N_CORES):
        b, half = core // 2, core % 2
        h0 = half * HALF
        o = res.results[core]["out"].astype(np.float32)  # [128, 16, 1024]
        o = o.reshape(2, 64, 16, 1024).transpose(1, 2, 0, 3).reshape(C, HALF, W)
        out[b, :, h0 : h0 + HALF, :] = o
    return out
